# revision 12
# baseline (speedup 1.0000x reference)
"""Multi-head self-attention (B=4, T=2048, D=1024, H=16) on 8 NeuronCores.

Sharding: batch x head-group. Core c handles batch b = c//2 and head group
g = c%2 (8 heads of 64 dims each). Host pre-transposes x and slices/
transposes the weights; each core computes its 8 heads' attention and a
partial output projection; host sums the two partials per batch and adds bo.

Per-core pipeline (all matmuls on TensorE, exp on ScalarE):
  v    = x @ WvT_aug + bv      (fp32r, augmented with a ones column per head
                                so attn@V also accumulates softmax sums)
  qT/kT = (Wq x^T + bq), per head-pair resident in SBUF  (fp32r)
  scoresT[k,q] = kT^T qT / 8   (two heads row-tiled concurrently, K=64)
  e = exp(scoresT)             (ScalarE, bf16 out)
  ctxT[dh,q] += (v|1)^T e      (bf16; row 64 = softmax denominator S)
  ctxT /= S                    (reciprocal + ones-matmul partition broadcast)
  out_partial = ctxT^T WoT     (fp32r) -> DMA from PSUM to DRAM
"""

from contextlib import ExitStack

import numpy as np

import concourse.bass as bass
import concourse.mybir as mybir
import concourse.tile as tile
from concourse import bacc
from concourse.bass_utils import run_bass_kernel_spmd

F32 = mybir.dt.float32
F32R = mybir.dt.float32r
BF16 = mybir.dt.bfloat16
EXP = mybir.ActivationFunctionType.Exp

B, T, D = 4, 2048, 1024
H, DH = 16, 64
G = 512            # head-group width (8 heads x 64)
GH = 8             # heads per group
P = 128
DK = D // P        # 8 contraction k-tiles for D
NQT = T // P       # 16 q/t tiles of 128
NQC = T // 512     # 4 q chunks of 512
NKT = T // P       # 16 key tiles of 128
VW = GH * (DH + 1)   # 520: v free width incl. ones columns
VC = VW // 2       # 260: v projection N-chunk (psum bank limit 512 fp32)


def r(ap):
    return ap.bitcast(F32R)


def emit_body(tc, io):
    nc = tc.nc
    xT, wq, wk, wv, wo, bq, bk, bv, out = (
        io["xT"], io["wq"], io["wk"], io["wv"], io["wo"],
        io["bq"], io["bk"], io["bv"], io["out"])

    xT_r = xT.rearrange("(i p) t -> p i t", p=P)     # [128, 8, 2048]

    with ExitStack() as ctx:
        E = ctx.enter_context
        constp = E(tc.tile_pool(name="const", bufs=1))
        wqkp = E(tc.tile_pool(name="wqk", bufs=1))
        wvop = E(tc.tile_pool(name="wvo", bufs=1))
        vp = E(tc.tile_pool(name="vsb", bufs=1))
        ctxp = E(tc.tile_pool(name="ctxsb", bufs=1))
        qkp = E(tc.tile_pool(name="qksb", bufs=4))
        xs_p = E(tc.tile_pool(name="xs", bufs=4))
        ep = E(tc.tile_pool(name="e", bufs=4))
        rp = E(tc.tile_pool(name="recip", bufs=4))

        # ---- constants / weights resident in SBUF ----
        ones = constp.tile([P, 64], F32, name="ones")
        nc.sync.dma_start(r(ones[:]), r(io["ones"]))
        bq_sb = constp.tile([P, 4], F32, name="bq_sb")
        nc.sync.dma_start(bq_sb[:], bq[:])
        bk_sb = constp.tile([P, 4], F32, name="bk_sb")
        nc.sync.dma_start(bk_sb[:], bk[:])
        bv_sb = constp.tile([P, VW], F32, name="bv_sb")
        nc.sync.dma_start(bv_sb[:], bv[:])

        wq_sb = wqkp.tile([P, DK, G], F32, name="wq_sb")
        nc.sync.dma_start(r(wq_sb[:]), r(wq.rearrange("(i p) m -> p i m", p=P)))
        wk_sb = wqkp.tile([P, DK, G], F32, name="wk_sb")
        nc.sync.dma_start(r(wk_sb[:]), r(wk.rearrange("(i p) m -> p i m", p=P)))
        wv_sb = wvop.tile([P, DK, VW], F32, name="wv_sb")
        nc.sync.dma_start(r(wv_sb[:]), r(wv.rearrange("(i p) m -> p i m", p=P)))
        wo_sb = wvop.tile([P, 4, D], F32, name="wo_sb")
        nc.sync.dma_start(r(wo_sb[:]), r(wo.rearrange("(i p) m -> p i m", p=P)))

        v_sb = vp.tile([P, NQT, VW], BF16, name="v_sb")
        ctx_sb = ctxp.tile([P, 4, T], F32, name="ctx_sb")

        # ---- phase 1: v projection (all 8 heads), natural [t, hd] layout ----
        with tc.tile_pool(name="psv", bufs=8, space="PSUM") as ps_v:
            for tg in range(4):                  # groups of 4 t-tiles (512 t)
                v_ps = [[ps_v.tile([P, VC], F32, tag="vps", name=f"vps_{tg}_{ti}_{vc}")
                         for vc in range(2)]
                        for ti in range(4)]
                for dk in range(DK):
                    xs = xs_p.tile([P, 512], F32, tag="xs")
                    nc.sync.dma_start(
                        r(xs[:]), r(xT_r[:, dk, tg * 512:(tg + 1) * 512]))
                    for ti in range(4):
                        for vc in range(2):
                            nc.tensor.matmul(
                                v_ps[ti][vc][:],
                                r(xs[:, ti * P:(ti + 1) * P]),
                                r(wv_sb[:, dk, vc * VC:(vc + 1) * VC]),
                                start=(dk == 0), stop=(dk == DK - 1))
                for ti in range(4):
                    for vc in range(2):
                        nc.vector.tensor_add(
                            v_sb[:, tg * 4 + ti, vc * VC:(vc + 1) * VC],
                            v_ps[ti][vc][:], bv_sb[:, vc * VC:(vc + 1) * VC])

        # ---- phases 2+3: per head-pair j ----
        with tc.tile_pool(name="psqk", bufs=2, space="PSUM") as ps_qk, \
             tc.tile_pool(name="pss", bufs=3, space="PSUM") as ps_s, \
             tc.tile_pool(name="psctx", bufs=2, space="PSUM") as ps_ctx, \
             tc.tile_pool(name="psbc", bufs=1, space="PSUM") as ps_bc:
            for j in range(4):
                qT = qkp.tile([P, T], F32, tag="qkT")
                kT = qkp.tile([P, T], F32, tag="qkT")
                for qc in range(NQC):
                    q_ps = ps_qk.tile([P, 512], F32, tag="qk")
                    k_ps = ps_qk.tile([P, 512], F32, tag="qk")
                    for dk in range(DK):
                        xs = xs_p.tile([P, 512], F32, tag="xs")
                        nc.sync.dma_start(
                            r(xs[:]), r(xT_r[:, dk, qc * 512:(qc + 1) * 512]))
                        nc.tensor.matmul(
                            q_ps[:], r(wq_sb[:, dk, j * P:(j + 1) * P]),
                            r(xs[:]), start=(dk == 0), stop=(dk == DK - 1))
                        nc.tensor.matmul(
                            k_ps[:], r(wk_sb[:, dk, j * P:(j + 1) * P]),
                            r(xs[:]), start=(dk == 0), stop=(dk == DK - 1))
                    nc.vector.tensor_scalar_add(
                        r(qT[:, qc * 512:(qc + 1) * 512]), q_ps[:],
                        bq_sb[:, j:j + 1])
                    nc.vector.tensor_scalar_add(
                        r(kT[:, qc * 512:(qc + 1) * 512]), k_ps[:],
                        bk_sb[:, j:j + 1])

                for qc in range(NQC):
                    qs = slice(qc * 512, (qc + 1) * 512)
                    ctxA = ps_ctx.tile([P, 512], F32, tag="ctx")
                    ctxB = ps_ctx.tile([P, 512], F32, tag="ctx")
                    for kt in range(NKT):
                        ks = slice(kt * P, (kt + 1) * P)
                        sA = ps_s.tile([P, 512], F32, tag="s")
                        sB = ps_s.tile([P, 512], F32, tag="s")
                        nc.tensor.matmul(
                            sA[:], r(kT[0:64, ks]), r(qT[0:64, qs]),
                            start=True, stop=True, tile_position=(0, 0))
                        nc.tensor.matmul(
                            sB[:], r(kT[64:128, ks]), r(qT[64:128, qs]),
                            start=True, stop=True, tile_position=(64, 0))
                        eA = ep.tile([P, 512], BF16, tag="e")
                        eB = ep.tile([P, 512], BF16, tag="e")
                        nc.scalar.activation(eA[:], sA[:], EXP, scale=0.125)
                        nc.scalar.activation(eB[:], sB[:], EXP, scale=0.125)
                        nc.tensor.matmul(
                            ctxA[:65], v_sb[:, kt, j * 130:j * 130 + 65], eA[:],
                            start=(kt == 0), stop=(kt == NKT - 1))
                        nc.tensor.matmul(
                            ctxB[:65], v_sb[:, kt, j * 130 + 65:j * 130 + 130],
                            eB[:], start=(kt == 0), stop=(kt == NKT - 1))
                    # softmax denominators -> reciprocal -> row broadcast
                    rcA = rp.tile([P, 512], F32, tag="rc")
                    rcB = rp.tile([P, 512], F32, tag="rc")
                    with nc.allow_low_precision(reason="fp32r rounding"):
                        nc.vector.reciprocal(r(rcA[64:65, :]), ctxA[64:65, :])
                        nc.vector.reciprocal(r(rcB[64:65, :]), ctxB[64:65, :])
                    bcA = ps_bc.tile([P, 512], F32, tag="bc")
                    nc.tensor.matmul(bcA[:64], r(ones[64:65, :]),
                                     r(rcA[64:65, :]), start=True, stop=True)
                    rbA = rp.tile([P, 512], F32, tag="rb")
                    nc.vector.tensor_copy(rbA[:64, :], bcA[:64, :])
                    bcB = ps_bc.tile([P, 512], F32, tag="bc")
                    nc.tensor.matmul(bcB[:64], r(ones[64:65, :]),
                                     r(rcB[64:65, :]), start=True, stop=True)
                    rbB = rp.tile([P, 512], F32, tag="rb")
                    nc.vector.tensor_copy(rbB[:64, :], bcB[:64, :])
                    # normalized ctxT: head A rows 0:64 direct; head B via DMA
                    # repartition to rows 64:128
                    nc.vector.tensor_mul(r(ctx_sb[0:64, j, qs]), ctxA[0:64, :],
                                         rbA[:64, :])
                    tmpB = rp.tile([P, 512], F32, tag="tmpB")
                    nc.vector.tensor_mul(tmpB[:64, :], ctxB[0:64, :],
                                         rbB[:64, :])
                    nc.sync.dma_start(r(ctx_sb[64:128, j, qs]), r(tmpB[:64, :]))

        # ---- phase 4: output projection (partial over this head group) ----
        with tc.tile_pool(name="pso", bufs=4, space="PSUM") as ps_o, \
             tc.tile_pool(name="osb", bufs=4) as op:
            for qt in range(NQT):
                for dc in range(2):
                    o_ps = ps_o.tile([P, 512], F32, tag="o")
                    for j in range(4):
                        nc.tensor.matmul(
                            o_ps[:], r(ctx_sb[:, j, qt * P:(qt + 1) * P]),
                            r(wo_sb[:, j, dc * 512:(dc + 1) * 512]),
                            start=(j == 0), stop=(j == 3))
                    o_sb = op.tile([P, 512], F32, tag="osb")
                    nc.vector.tensor_copy(o_sb[:], o_ps[:])
                    nc.sync.dma_start(
                        out[qt * P:(qt + 1) * P, dc * 512:(dc + 1) * 512],
                        o_sb[:])


def build(loop_k: int = 1):
    nc = bacc.Bacc("TRN2", target_bir_lowering=False, debug=False)
    io = {
        "xT": nc.dram_tensor("xT", [D, T], F32, kind="ExternalInput").ap(),
        "wq": nc.dram_tensor("wq", [D, G], F32, kind="ExternalInput").ap(),
        "wk": nc.dram_tensor("wk", [D, G], F32, kind="ExternalInput").ap(),
        "wv": nc.dram_tensor("wv", [D, VW], F32, kind="ExternalInput").ap(),
        "wo": nc.dram_tensor("wo", [G, D], F32, kind="ExternalInput").ap(),
        "bq": nc.dram_tensor("bq", [P, 4], F32, kind="ExternalInput").ap(),
        "bk": nc.dram_tensor("bk", [P, 4], F32, kind="ExternalInput").ap(),
        "bv": nc.dram_tensor("bv", [P, VW], F32, kind="ExternalInput").ap(),
        "ones": nc.dram_tensor("ones", [P, 64], F32, kind="ExternalInput").ap(),
        "out": nc.dram_tensor("out", [T, D], F32, kind="ExternalOutput").ap(),
    }
    with tile.TileContext(nc) as tc:
        if loop_k == 1:
            emit_body(tc, io)
        else:
            with tc.For_i(0, loop_k, 1):
                emit_body(tc, io)
    nc.compile()
    return nc


def prep_inputs(x, Wq, bq, Wk, bk, Wv, bv, Wo, bo):
    """Host-side sharding: returns in_maps for cores 0..7."""
    f = np.float32
    in_maps = []
    for c in range(8):
        b, g = c // 2, c % 2
        gs = slice(g * G, (g + 1) * G)
        wv_aug = np.zeros((D, VW), f)
        bv_aug = np.zeros((VW,), f)
        wv_g = np.ascontiguousarray(Wv[gs, :].T)        # [D, 512]
        for h in range(GH):
            wv_aug[:, h * 65:h * 65 + 64] = wv_g[:, h * 64:(h + 1) * 64]
            bv_aug[h * 65:h * 65 + 64] = bv[gs][h * 64:(h + 1) * 64]
            bv_aug[h * 65 + 64] = 1.0
        in_maps.append({
            "xT": np.ascontiguousarray(np.asarray(x[b]).T),
            "wq": np.ascontiguousarray(Wq[gs, :].T),
            "wk": np.ascontiguousarray(Wk[gs, :].T),
            "wv": wv_aug,
            "wo": np.ascontiguousarray(Wo[:, gs].T),
            "bq": np.ascontiguousarray(bq[gs].reshape(4, P).T),
            "bk": np.ascontiguousarray(bk[gs].reshape(4, P).T),
            "bv": np.broadcast_to(bv_aug, (P, VW)).copy(),
            "ones": np.ones((P, 64), f),
        })
    return in_maps


def gather_output(results, bo):
    out = np.empty((B, T, D), np.float32)
    for b in range(B):
        out[b] = (results[2 * b]["out"] + results[2 * b + 1]["out"]
                  + np.asarray(bo)[None, :])
    return out


_nc_cache = {}


def kernel(x, Wq, bq, Wk, bk, Wv, bv, Wo, bo):
    if "nc" not in _nc_cache:
        _nc_cache["nc"] = build()
    nc = _nc_cache["nc"]
    in_maps = prep_inputs(x, Wq, bq, Wk, bk, Wv, bv, Wo, bo)
    res = run_bass_kernel_spmd(nc, in_maps, list(range(8)))
    return gather_output(res.results, bo)


# revision 13
# speedup vs baseline: 1.2901x; 1.2901x over previous
"""Multi-head self-attention (B=4, T=2048, D=1024, H=16) on 8 NeuronCores.

Sharding: batch x head-group. Core c handles batch b = c//2 and head group
g = c%2 (8 heads of 64 dims each). Host pre-transposes x and slices/
transposes the weights; each core computes its 8 heads' attention and a
partial output projection; host sums the two partials per batch and adds bo.

Per-core pipeline (matmuls on TensorE, exp on ScalarE):
  v    = x @ WvT_aug + bv       (bf16, augmented with a ones column per head
                                 so attn@V also accumulates softmax sums)
  qT/kT = (W x^T + b), all 4 head-pairs in one x pass   (bf16)
  scoresT[k,q] = kT^T qT / 8    (two heads row-tiled concurrently, K=64)
  e = exp(scoresT)              (ScalarE, bf16 out)
  ctxT[dh,q] += (v|1)^T e       (bf16; row 64 = softmax denominator S)
  ctxT /= S                     (reciprocal + ones-matmul partition bcast, f32r)
  out_partial = ctxT^T WoT      (f32r for final precision)

HW-calibrated dtype costs (N=512 moving dim): bf16 1 cyc/row, fp32r 2,
fp32 4. Row-tiled K=64 pairs run concurrently (~181 ns/matmul measured).
"""

from contextlib import ExitStack

import numpy as np
import ml_dtypes

import concourse.bass as bass
import concourse.mybir as mybir
import concourse.tile as tile
from concourse import bacc
from concourse.bass_utils import run_bass_kernel_spmd

F32 = mybir.dt.float32
F32R = mybir.dt.float32r
BF16 = mybir.dt.bfloat16
EXP = mybir.ActivationFunctionType.Exp

B, T, D = 4, 2048, 1024
H, DH = 16, 64
G = 512            # head-group width (8 heads x 64)
GH = 8             # heads per group
P = 128
DK = D // P        # 8 contraction k-tiles for D
NQT = T // P       # 16 q/t tiles of 128
NQC = T // 512     # 4 q chunks of 512
NKT = T // P       # 16 key tiles of 128
VW = GH * (DH + 1)   # 520: v free width incl. ones columns
VC = VW // 2       # 260: v projection N-chunk (psum bank limit 512 fp32)


def r(ap):
    return ap.bitcast(F32R)


def emit_body(tc, io):
    nc = tc.nc
    xT, wq, wk, wv, wo, bq, bk, bv, out = (
        io["xT"], io["wq"], io["wk"], io["wv"], io["wo"],
        io["bq"], io["bk"], io["bv"], io["out"])

    xT_r = xT.rearrange("(i p) t -> p i t", p=P)     # [128, 8, 2048] bf16

    with ExitStack() as ctx:
        E = ctx.enter_context
        constp = E(tc.tile_pool(name="const", bufs=1))
        wqkp = E(tc.tile_pool(name="wqk", bufs=1))
        wvop = E(tc.tile_pool(name="wvo", bufs=1))
        vp = E(tc.tile_pool(name="vsb", bufs=1))
        ctxp = E(tc.tile_pool(name="ctxsb", bufs=1))
        qkp = E(tc.tile_pool(name="qksb", bufs=1))
        xs_p = E(tc.tile_pool(name="xs", bufs=4))
        ep = E(tc.tile_pool(name="e", bufs=6))
        rp = E(tc.tile_pool(name="recip", bufs=4))

        # ---- constants / weights resident in SBUF ----
        ones = constp.tile([P, 64], F32, name="ones")
        nc.sync.dma_start(r(ones[:]), r(io["ones"]))
        bq_sb = constp.tile([P, 4], F32, name="bq_sb")
        nc.sync.dma_start(bq_sb[:], bq[:])
        bk_sb = constp.tile([P, 4], F32, name="bk_sb")
        nc.sync.dma_start(bk_sb[:], bk[:])
        bv_sb = constp.tile([P, VW], F32, name="bv_sb")
        nc.sync.dma_start(bv_sb[:], bv[:])

        wq_sb = wqkp.tile([P, DK, G], BF16, name="wq_sb")
        nc.sync.dma_start(wq_sb[:], wq.rearrange("(i p) m -> p i m", p=P))
        wk_sb = wqkp.tile([P, DK, G], BF16, name="wk_sb")
        nc.sync.dma_start(wk_sb[:], wk.rearrange("(i p) m -> p i m", p=P))
        wv_sb = wvop.tile([P, DK, VW], BF16, name="wv_sb")
        nc.sync.dma_start(wv_sb[:], wv.rearrange("(i p) m -> p i m", p=P))
        wo_sb = wvop.tile([P, 4, D], F32, name="wo_sb")
        nc.sync.dma_start(r(wo_sb[:]), r(wo.rearrange("(i p) m -> p i m", p=P)))

        v_sb = vp.tile([P, NQT, VW], BF16, name="v_sb")
        ctx_sb = ctxp.tile([P, 4, T], F32, name="ctx_sb")
        qT_sb = qkp.tile([P, 4, T], BF16, name="qT_sb")
        kT_sb = qkp.tile([P, 4, T], BF16, name="kT_sb")

        # ---- phase 1: v projection (all 8 heads), natural [t, hd] layout ----
        with tc.tile_pool(name="psv", bufs=8, space="PSUM") as ps_v:
            for tg in range(4):                  # groups of 4 t-tiles (512 t)
                v_ps = [[ps_v.tile([P, VC], F32, tag="vps",
                                   name=f"vps_{tg}_{ti}_{vc}")
                         for vc in range(2)] for ti in range(4)]
                for dk in range(DK):
                    xs = xs_p.tile([P, 512], BF16, tag="xs")
                    nc.sync.dma_start(
                        xs[:], xT_r[:, dk, tg * 512:(tg + 1) * 512])
                    for ti in range(4):
                        for vc in range(2):
                            nc.tensor.matmul(
                                v_ps[ti][vc][:],
                                xs[:, ti * P:(ti + 1) * P],
                                wv_sb[:, dk, vc * VC:(vc + 1) * VC],
                                start=(dk == 0), stop=(dk == DK - 1))
                for ti in range(4):
                    for vc in range(2):
                        nc.vector.tensor_add(
                            v_sb[:, tg * 4 + ti, vc * VC:(vc + 1) * VC],
                            v_ps[ti][vc][:], bv_sb[:, vc * VC:(vc + 1) * VC])

        # ---- phase 2: qT/kT for all 4 head-pairs in one x pass ----
        with tc.tile_pool(name="psqk", bufs=8, space="PSUM") as ps_qk:
            for qc in range(NQC):
                qs = slice(qc * 512, (qc + 1) * 512)
                q_ps = [ps_qk.tile([P, 512], F32, tag="qk", name=f"qps{qc}_{j}")
                        for j in range(4)]
                k_ps = [ps_qk.tile([P, 512], F32, tag="qk", name=f"kps{qc}_{j}")
                        for j in range(4)]
                for dk in range(DK):
                    xs = xs_p.tile([P, 512], BF16, tag="xs")
                    nc.sync.dma_start(xs[:], xT_r[:, dk, qs])
                    for j in range(4):
                        nc.tensor.matmul(
                            q_ps[j][:], wq_sb[:, dk, j * P:(j + 1) * P],
                            xs[:], start=(dk == 0), stop=(dk == DK - 1))
                        nc.tensor.matmul(
                            k_ps[j][:], wk_sb[:, dk, j * P:(j + 1) * P],
                            xs[:], start=(dk == 0), stop=(dk == DK - 1))
                for j in range(4):
                    nc.vector.tensor_scalar_add(
                        qT_sb[:, j, qs], q_ps[j][:], bq_sb[:, j:j + 1])
                    nc.vector.tensor_scalar_add(
                        kT_sb[:, j, qs], k_ps[j][:], bk_sb[:, j:j + 1])

        # ---- phase 3: attention per head-pair ----
        with tc.tile_pool(name="pss", bufs=4, space="PSUM") as ps_s, \
             tc.tile_pool(name="psctx", bufs=2, space="PSUM") as ps_ctx, \
             tc.tile_pool(name="psbc", bufs=2, space="PSUM") as ps_bc:
            for j in range(4):
                for qc in range(NQC):
                    qs = slice(qc * 512, (qc + 1) * 512)
                    ctxA = ps_ctx.tile([P, 512], F32, tag="ctx")
                    ctxB = ps_ctx.tile([P, 512], F32, tag="ctx")
                    for kt in range(NKT):
                        ks = slice(kt * P, (kt + 1) * P)
                        sA = ps_s.tile([P, 512], F32, tag="s")
                        sB = ps_s.tile([P, 512], F32, tag="s")
                        nc.tensor.matmul(
                            sA[:], kT_sb[0:64, j, ks], qT_sb[0:64, j, qs],
                            start=True, stop=True, tile_position=(0, 0))
                        nc.tensor.matmul(
                            sB[:], kT_sb[64:128, j, ks], qT_sb[64:128, j, qs],
                            start=True, stop=True, tile_position=(64, 0))
                        eA = ep.tile([P, 512], BF16, tag="e")
                        eB = ep.tile([P, 512], BF16, tag="e")
                        nc.scalar.activation(eA[:], sA[:], EXP, scale=0.125)
                        nc.scalar.activation(eB[:], sB[:], EXP, scale=0.125)
                        nc.tensor.matmul(
                            ctxA[:65], v_sb[:, kt, j * 130:j * 130 + 65],
                            eA[:], start=(kt == 0), stop=(kt == NKT - 1))
                        nc.tensor.matmul(
                            ctxB[:65], v_sb[:, kt, j * 130 + 65:j * 130 + 130],
                            eB[:], start=(kt == 0), stop=(kt == NKT - 1))
                    # softmax denominators -> reciprocal -> row broadcast
                    rcA = rp.tile([P, 512], F32, tag="rc")
                    rcB = rp.tile([P, 512], F32, tag="rc")
                    with nc.allow_low_precision(reason="fp32r rounding"):
                        nc.vector.reciprocal(r(rcA[64:65, :]), ctxA[64:65, :])
                        nc.vector.reciprocal(r(rcB[64:65, :]), ctxB[64:65, :])
                    bcA = ps_bc.tile([P, 512], F32, tag="bc")
                    nc.tensor.matmul(bcA[:64], r(ones[64:65, :]),
                                     r(rcA[64:65, :]), start=True, stop=True)
                    rbA = rp.tile([P, 512], F32, tag="rb")
                    nc.vector.tensor_copy(rbA[:64, :], bcA[:64, :])
                    bcB = ps_bc.tile([P, 512], F32, tag="bc")
                    nc.tensor.matmul(bcB[:64], r(ones[64:65, :]),
                                     r(rcB[64:65, :]), start=True, stop=True)
                    rbB = rp.tile([P, 512], F32, tag="rb")
                    nc.vector.tensor_copy(rbB[:64, :], bcB[:64, :])
                    # normalized ctxT: head A rows 0:64 direct; head B via DMA
                    # repartition to rows 64:128
                    nc.vector.tensor_mul(r(ctx_sb[0:64, j, qs]), ctxA[0:64, :],
                                         rbA[:64, :])
                    tmpB = rp.tile([P, 512], F32, tag="tmpB")
                    nc.vector.tensor_mul(tmpB[:64, :], ctxB[0:64, :],
                                         rbB[:64, :])
                    nc.sync.dma_start(r(ctx_sb[64:128, j, qs]), r(tmpB[:64, :]))

        # ---- phase 4: output projection (partial over this head group) ----
        with tc.tile_pool(name="pso", bufs=4, space="PSUM") as ps_o, \
             tc.tile_pool(name="osb", bufs=4) as op:
            for qt in range(NQT):
                for dc in range(2):
                    o_ps = ps_o.tile([P, 512], F32, tag="o")
                    for j in range(4):
                        nc.tensor.matmul(
                            o_ps[:], r(ctx_sb[:, j, qt * P:(qt + 1) * P]),
                            r(wo_sb[:, j, dc * 512:(dc + 1) * 512]),
                            start=(j == 0), stop=(j == 3))
                    o_sb = op.tile([P, 512], F32, tag="osb")
                    nc.vector.tensor_copy(o_sb[:], o_ps[:])
                    nc.sync.dma_start(
                        out[qt * P:(qt + 1) * P, dc * 512:(dc + 1) * 512],
                        o_sb[:])


def build(loop_k: int = 1):
    nc = bacc.Bacc("TRN2", target_bir_lowering=False, debug=False)
    io = {
        "xT": nc.dram_tensor("xT", [D, T], BF16, kind="ExternalInput").ap(),
        "wq": nc.dram_tensor("wq", [D, G], BF16, kind="ExternalInput").ap(),
        "wk": nc.dram_tensor("wk", [D, G], BF16, kind="ExternalInput").ap(),
        "wv": nc.dram_tensor("wv", [D, VW], BF16, kind="ExternalInput").ap(),
        "wo": nc.dram_tensor("wo", [G, D], F32, kind="ExternalInput").ap(),
        "bq": nc.dram_tensor("bq", [P, 4], F32, kind="ExternalInput").ap(),
        "bk": nc.dram_tensor("bk", [P, 4], F32, kind="ExternalInput").ap(),
        "bv": nc.dram_tensor("bv", [P, VW], F32, kind="ExternalInput").ap(),
        "ones": nc.dram_tensor("ones", [P, 64], F32, kind="ExternalInput").ap(),
        "out": nc.dram_tensor("out", [T, D], F32, kind="ExternalOutput").ap(),
    }
    with tile.TileContext(nc) as tc:
        if loop_k == 1:
            emit_body(tc, io)
        else:
            with tc.For_i(0, loop_k, 1):
                emit_body(tc, io)
    nc.compile()
    return nc


def prep_inputs(x, Wq, bq, Wk, bk, Wv, bv, Wo, bo):
    """Host-side sharding: returns in_maps for cores 0..7."""
    f = np.float32
    bf = ml_dtypes.bfloat16
    in_maps = []
    for c in range(8):
        b, g = c // 2, c % 2
        gs = slice(g * G, (g + 1) * G)
        wv_aug = np.zeros((D, VW), f)
        bv_aug = np.zeros((VW,), f)
        wv_g = np.ascontiguousarray(Wv[gs, :].T)        # [D, 512]
        for h in range(GH):
            wv_aug[:, h * 65:h * 65 + 64] = wv_g[:, h * 64:(h + 1) * 64]
            bv_aug[h * 65:h * 65 + 64] = bv[gs][h * 64:(h + 1) * 64]
            bv_aug[h * 65 + 64] = 1.0
        in_maps.append({
            "xT": np.ascontiguousarray(np.asarray(x[b]).T).astype(bf),
            "wq": np.ascontiguousarray(Wq[gs, :].T).astype(bf),
            "wk": np.ascontiguousarray(Wk[gs, :].T).astype(bf),
            "wv": wv_aug.astype(bf),
            "wo": np.ascontiguousarray(Wo[:, gs].T),
            "bq": np.ascontiguousarray(bq[gs].reshape(4, P).T),
            "bk": np.ascontiguousarray(bk[gs].reshape(4, P).T),
            "bv": np.broadcast_to(bv_aug, (P, VW)).copy(),
            "ones": np.ones((P, 64), f),
        })
    return in_maps


def gather_output(results, bo):
    out = np.empty((B, T, D), np.float32)
    for b in range(B):
        out[b] = (results[2 * b]["out"] + results[2 * b + 1]["out"]
                  + np.asarray(bo)[None, :])
    return out


_nc_cache = {}


def kernel(x, Wq, bq, Wk, bk, Wv, bv, Wo, bo):
    if "nc" not in _nc_cache:
        _nc_cache["nc"] = build()
    nc = _nc_cache["nc"]
    in_maps = prep_inputs(x, Wq, bq, Wk, bk, Wv, bv, Wo, bo)
    res = run_bass_kernel_spmd(nc, in_maps, list(range(8)))
    return gather_output(res.results, bo)
